# revision 34
# baseline (speedup 1.0000x reference)
"""Low-pass FFT filtering kernel for Trainium2 (8 NeuronCores).

Math: reference does, per (batch b, channel i), with X = x[b,:,:,i] (256x256):
    out_i = irfft(rfft(X, axis=0) * mask) + irfft(rfft(X, axis=1) * mask)
with mask keeping rfft modes 0..15 (ortho norm). That filter is an orthogonal
projection P = W @ W.T where W [256, 31] is the orthonormal basis
{1/sqrt(n), sqrt(2/n)cos(2pi k t/n), -sqrt(2/n)sin(2pi k t/n)}_{k=1..15}.
So  out_i = P @ X_i + X_i @ P = W @ (W.T @ X_i) + (X_i @ W) @ W.T.

Device schedule (per core = one batch, channel-major layouts):
  C = W.T @ Xcm   [31, I*N]   (Xcm = x[b] as [m, (i, n)])
  D = W.T @ Xt    [31, I*M]   (Xt  = x[b] as [n, (i, m)], host-transposed)
  out[m-tile, n'] per (i, j):  single K=63 matmul with
     lhsT = [Wt_j ; 0 ; D_i,j]  (63 x 128),  rhs = [C_i ; 0 ; Wt] (63 x 256)
  which accumulates both terms in one PSUM pass.

v2 scheduling notes:
  - Lg/Rg are persistent SBUF buffers (3 each, rotated per chunk); their
    constant rows (tiled W^T and the zero row) are DMA'd once at startup
    instead of re-loaded from HBM per chunk.
  - One DMA per chunk per input stream using a [128, 2, w] pattern that
    covers both 128-row halves; xc rides the SP HWDGE ring, xt the ACT ring.
  - Outputs are issued per chunk on the GpSimd SWDGE ring (and the last
    chunk's halves on the SP ring tail) so they overlap the input stream
    instead of queueing behind it.
  - Phase-1 runs weight-major (all W-top matmuls of a chunk, then all
    W-bottom) to amortize LDWEIGHTS.
  - PSUM->SBUF traffic is spread over ACT (C rows), DVE (D rows) and
    ACT/DVE/Pool round-robin for the output casts.
Inputs/weights are fp16 on device; accumulation is fp32 in PSUM; output is
staged fp16 and upcast to fp32 on host (rel err ~7e-4 end to end).
Sharding: batch b -> core b (8 cores, no communication).
"""

import os
import sys
import types

import numpy as np

import concourse.bass as bass
import concourse.bacc as bacc
import concourse.tile as tile
from concourse import mybir
from concourse.bass_utils import run_bass_kernel_spmd

B, M, N, I = 8, 256, 256, 32
KMAX = 16           # modes kept: 0..15
R = 2 * KMAX - 1    # 31 real basis vectors
FREE = I * N        # 8192
CCOLS = 2048        # max chunk width (8 channels)
F32 = mybir.dt.float32
F16 = mybir.dt.float16
NPDT = np.float16

WIDTHS = [512, 1024, 1536, 1536, 1536, 1024, 512, 512]

LAST_RESULTS = None  # BassKernelResults of the most recent run (for test.py)


def _ensure_ntff_hook():
    """Provide antenv.axon_hooks if the image lacks it, so trace=True works."""
    try:
        from antenv.axon_hooks import get_axon_ntff_profile_hook  # noqa: F401
        return
    except ImportError:
        pass
    try:
        from trn_agent_boot.trn_boot import _ntff_profile_via_ctypes
        hook = _ntff_profile_via_ctypes("/opt/axon/libaxon_pjrt.so")
    except Exception:
        hook = None
    mod = types.ModuleType("antenv.axon_hooks")
    _state = {"hook": hook}
    mod.get_axon_ntff_profile_hook = lambda: _state["hook"]
    mod.set_axon_ntff_profile_hook = lambda h: _state.update(hook=h)
    sys.modules["antenv.axon_hooks"] = mod
    try:
        import antenv
        antenv.axon_hooks = mod
    except ImportError:
        pass


def _basis():
    t = np.arange(N)
    cols = [np.ones(N) / np.sqrt(N)]
    for k in range(1, KMAX):
        cols.append(np.sqrt(2.0 / N) * np.cos(2 * np.pi * k * t / N))
        cols.append(-np.sqrt(2.0 / N) * np.sin(2 * np.pi * k * t / N))
    return np.stack(cols, axis=1).astype(np.float32)  # [256, 31]


def _build_nc():
    nc = bacc.Bacc("TRN2", target_bir_lowering=False, debug=False,
                   enable_asserts=False, num_devices=8)

    xc = nc.declare_dram_parameter("xc", [M, FREE], F16, isOutput=False)
    xt = nc.declare_dram_parameter("xt", [N, I * M], F16, isOutput=False)
    w2 = nc.declare_dram_parameter("w2", [128, 2 * R], F16, isOutput=False)
    wz = nc.declare_dram_parameter("wz", [R + 1, CCOLS], F16, isOutput=False)
    zw = nc.declare_dram_parameter("zw", [R + 1, CCOLS], F16, isOutput=False)
    out = nc.declare_dram_parameter("out", [M, FREE], F16, isOutput=True)

    starts = [0]
    for w_ in WIDTHS[:-1]:
        starts.append(starts[-1] + w_)

    with tile.TileContext(nc) as tc:
        with (
            tc.tile_pool(name="const", bufs=1) as constp,
            tc.tile_pool(name="xin", bufs=len(WIDTHS)) as xin,
            tc.tile_pool(name="oput", bufs=len(WIDTHS)) as outp,
            tc.tile_pool(name="pcd", bufs=3, space=bass.MemorySpace.PSUM) as pcdp,
            tc.tile_pool(name="p2", bufs=5, space=bass.MemorySpace.PSUM) as p2p,
        ):
            w2sb = constp.tile([128, 2 * R], F16)
            nc.sync.dma_start(out=w2sb[:], in_=w2[:])

            # Each L/R buffer has a low half (partitions 0:63) and a high
            # half (64:127): consecutive f-tiles alternate halves so their
            # phase-2 K=63 matmuls co-issue on the two PE row halves (the
            # row tile position must equal the stationary's start partition).
            NLR = 2
            Lgs = [constp.tile([128, CCOLS], F16, name=f"Lg{k}")
                   for k in range(NLR)]
            Rgs = [constp.tile([128, CCOLS], F16, name=f"Rg{k}")
                   for k in range(NLR)]

            # all input DMAs are issued up front so the SP/ACT sequencers
            # never block input descriptor-gen behind an output that is
            # waiting on casts. Chunk 0 (x and t) rides SP first so the PE
            # starts ASAP; outputs go on the SP ring tail, FIFO behind all
            # inputs = strict input priority on HBM bandwidth.
            xgs, tgs = [], []
            for g, (c0, w) in enumerate(zip(starts, WIDTHS)):
                xg = xin.tile([128, 2, w], F16, tag="x", name=f"xg{g}")
                tg = xin.tile([128, 2, w], F16, tag="t", name=f"tg{g}")
                xgs.append(xg)
                tgs.append(tg)
            # NOTE: the GpSimd SWDGE queue is fed by software descriptor-gen
            # and, while active, drags the whole 16-engine DMA pool down to
            # ~250GB/s (measured) -- never put anything on it. Everything
            # rides ONE HWDGE queue (SP) in hand-crafted FIFO order: the
            # FIFO then delivers each chunk's x+t just-in-time, consts early,
            # and drains each chunk's output right behind the input of the
            # chunk two ahead -- strict pipeline pacing with zero cross-queue
            # competition for the 16 DMA engines.
            def load_x(g):
                c0, w = starts[g], WIDTHS[g]
                nc.sync.dma_start(
                    out=xgs[g][:],
                    in_=xc[:, c0:c0 + w].rearrange("(h p) c -> p h c", h=2))

            def load_t(g, eng):
                c0, w = starts[g], WIDTHS[g]
                eng.dma_start(
                    out=tgs[g][:],
                    in_=xt[:, c0:c0 + w].rearrange("(h p) c -> p h c", h=2))

            # x-stream + outputs on the SP HWDGE queue; t-stream + consts on
            # the ACT HWDGE queue. Each queue is a hand-ordered FIFO that
            # delivers chunk g's input just-in-time and drains chunk g's
            # output behind the input of chunk g+IN_LEAD; two queues keep the
            # 16 DMA engines busy across per-instruction FIFO boundaries.
            IN_LEAD = 4  # chunks of input issued ahead of the pipeline
            load_x(0)
            load_t(0, nc.sync)   # chunk-0 t on SP: ACT's queue opens later
            load_t(1, nc.scalar)
            for k in range(NLR):
                for h in (0, 64):
                    nc.scalar.dma_start(out=Lgs[k][h:h + 32, :], in_=wz[:])
                    nc.scalar.dma_start(out=Rgs[k][h + 31:h + 63, :], in_=zw[:])
            for g in range(1, IN_LEAD):
                load_x(g)
            for g in range(2, IN_LEAD):
                load_t(g, nc.scalar)

            ogs = [outp.tile([128, 2, w], F16, tag="o", name=f"og{g}")
                   for g, w in enumerate(WIDTHS)]

            # global f-tile pipeline: each f-tile is 512 cols = one channel
            # pair; phase 2 of pair s runs two f-tiles behind phase 1 of
            # pair s. tile_set_cur_wait pins this order in the scheduler so
            # the in-order PE never queues input-blocked phase-1 work ahead
            # of ready phase-2 work.
            fl = []
            for g, (c0, w) in enumerate(zip(starts, WIDTHS)):
                for f in range(w // 512):
                    fl.append((g, f))

            def do_p1_pair(ftiles):
                # two f-tiles share one [128, 512] PSUM bank: C1@0, D1@32,
                # C2@64, D2@96 -- explicit tile_position unlocks the 96-base
                # quadrant so 4 PE column groups stream concurrently.
                pc = pcdp.tile([128, 512], F32, tag="pcd", name="pc")
                units = []
                for u, (g, f) in enumerate(ftiles):
                    fsl = slice(f * 512, (f + 1) * 512)
                    units.append((64 * u, g, fsl))
                for half, W_cols in ((0, slice(0, R)), (1, slice(R, 2 * R))):
                    st = half == 0
                    sp = half == 1
                    for r0, g, fsl in units:
                        nc.tensor.matmul(pc[r0:r0 + R, :], w2sb[:, W_cols],
                                         xgs[g][:, half, fsl],
                                         start=st, stop=sp,
                                         tile_position=(0, r0))
                        nc.tensor.matmul(pc[r0 + 32:r0 + 63, :],
                                         w2sb[:, W_cols], tgs[g][:, half, fsl],
                                         start=st, stop=sp,
                                         tile_position=(0, r0 + 32))
                for u, (r0, g, fsl) in enumerate(units):
                    Lg, Rg = Lgs[g % NLR], Rgs[g % NLR]
                    hb = 64 * u  # L/R half for this f-tile (pair parity)
                    # C rows on ACT, D rows on DVE
                    nc.scalar.copy(Rg[hb:hb + R, fsl], pc[r0:r0 + R, :])
                    nc.vector.tensor_copy(Lg[hb + 32:hb + 63, fsl],
                                          pc[r0 + 32:r0 + 63, :])

            cast_engines = [lambda o, i: nc.vector.tensor_copy(o, i),
                            lambda o, i: nc.scalar.copy(o, i)]

            def ship(g):
                # whole chunk cast: ship it, then queue the input of the
                # chunk IN_LEAD ahead right behind it in the FIFO
                c0, w = starts[g], WIDTHS[g]
                dst = out[:, c0:c0 + w].rearrange("(h p) c -> p h c", h=2)
                nc.sync.dma_start(out=dst, in_=ogs[g][:])
                if g + IN_LEAD < len(WIDTHS):
                    load_x(g + IN_LEAD)
                    load_t(g + IN_LEAD, nc.scalar)

            def do_p2_pair(ftiles):
                # the two f-tiles' K=63 matmuls co-issue on PE row halves
                # (tile_position row base 0 / 64)
                for j in range(2):
                    p2s = []
                    for u, (g, f) in enumerate(ftiles):
                        p2 = p2p.tile([128, 2 * N], F32, tag="p2", name="p2")
                        p2s.append(p2)
                    for s in range(2):
                        for u, (g, f) in enumerate(ftiles):
                            Lg, Rg = Lgs[g % NLR], Rgs[g % NLR]
                            hb = 64 * u
                            il = 2 * f + s
                            csl = slice(il * N, (il + 1) * N)
                            jsl = slice(il * N + j * 128,
                                        il * N + (j + 1) * 128)
                            nc.tensor.matmul(p2s[u][:, s * N:(s + 1) * N],
                                             Lg[hb:hb + 63, jsl],
                                             Rg[hb:hb + 63, csl],
                                             start=True, stop=True)
                    for u, (g, f) in enumerate(ftiles):
                        cast_engines[(j + u) % 2](
                            ogs[g][:, j, 2 * f * N:(2 * f + 2) * N], p2s[u][:])
                for g, f in ftiles:
                    if f == WIDTHS[g] // 512 - 1:
                        ship(g)

            # phase-1 advances in f-tile pairs; phase-2 lags one pair
            groups = [fl[i:i + 2] for i in range(0, len(fl), 2)]
            for q in range(len(groups) + 1):
                tc.tile_set_cur_wait(0.01 * (q + 1))
                if q < len(groups):
                    do_p1_pair(groups[q])
                if q >= 1:
                    tc.tile_set_cur_wait(0.01 * (q + 1) + 0.005)
                    do_p2_pair(groups[q - 1])

    nc.finalize()
    return nc


_NC = None


def kernel(x: np.ndarray) -> np.ndarray:
    global _NC, LAST_RESULTS
    x = np.asarray(x)
    assert x.shape == (B, M, N, I), x.shape

    W = _basis().astype(NPDT)          # [256, 31]
    Wt = W.T.copy()                    # [31, 256]
    w2_np = np.concatenate([W[0:128, :], W[128:256, :]], axis=1)  # [128, 62]
    wtile = np.tile(Wt, (1, CCOLS // N))                          # [31, 2048]
    wz_np = np.concatenate([wtile, np.zeros((1, CCOLS), NPDT)], axis=0)
    zw_np = np.concatenate([np.zeros((1, CCOLS), NPDT), wtile], axis=0)

    if _NC is None:
        _NC = _build_nc()

    xq = np.asarray(x, dtype=NPDT)
    in_maps = []
    for b in range(B):
        xcm = np.ascontiguousarray(xq[b].transpose(0, 2, 1)).reshape(M, FREE)
        xtm = np.ascontiguousarray(xq[b].transpose(1, 2, 0)).reshape(N, I * M)
        in_maps.append({
            "xc": xcm, "xt": xtm,
            "w2": w2_np, "wz": wz_np, "zw": zw_np,
        })

    trace = bool(int(os.environ.get("KERNEL_TRACE", "0")))
    if trace:
        _ensure_ntff_hook()
    last_err = None
    for attempt in range(3):
        try:
            LAST_RESULTS = run_bass_kernel_spmd(_NC, in_maps, list(range(B)),
                                                trace=trace and attempt == 0)
            break
        except Exception as e:  # rare transient NRT_EXEC_UNIT_UNRECOVERABLE
            last_err = e
            import time as _time
            _time.sleep(2.0)
            try:
                import jax
                jax.clear_caches()
                jax.extend.backend.clear_backends()
            except Exception:
                pass
    else:
        raise last_err

    out = np.empty((B, M, N, I), np.float32)
    for b in range(B):
        dev = LAST_RESULTS.results[b]["out"].astype(np.float32).reshape(M, I, N)
        out[b] = dev.transpose(0, 2, 1)
    return out


# revision 53
# speedup vs baseline: 1.0101x; 1.0101x over previous
"""Low-pass FFT filtering kernel for Trainium2 (8 NeuronCores).

Math: reference does, per (batch b, channel i), with X = x[b,:,:,i] (256x256):
    out_i = irfft(rfft(X, axis=0) * mask) + irfft(rfft(X, axis=1) * mask)
with mask keeping rfft modes 0..15 (ortho norm). That filter is an orthogonal
projection P = W @ W.T where W [256, 31] is the orthonormal basis
{1/sqrt(n), sqrt(2/n)cos(2pi k t/n), -sqrt(2/n)sin(2pi k t/n)}_{k=1..15}.
So  out_i = P @ X_i + X_i @ P = W @ (W.T @ X_i) + (X_i @ W) @ W.T.

Device schedule (per core = one batch, channel-major layouts):
  C = W.T @ Xcm   [31, I*N]   (Xcm = x[b] as [m, (i, n)])
  D = W.T @ Xt    [31, I*M]   (Xt  = x[b] as [n, (i, m)], host-transposed)
  out[m-tile, n'] per (i, j):  single K=63 matmul with
     lhsT = [Wt_j ; 0 ; D_i,j]  (63 x 128),  rhs = [C_i ; 0 ; Wt] (63 x 256)
  which accumulates both terms in one PSUM pass.

v2 scheduling notes:
  - Lg/Rg are persistent SBUF buffers (3 each, rotated per chunk); their
    constant rows (tiled W^T and the zero row) are DMA'd once at startup
    instead of re-loaded from HBM per chunk.
  - One DMA per chunk per input stream using a [128, 2, w] pattern that
    covers both 128-row halves; xc rides the SP HWDGE ring, xt the ACT ring.
  - Outputs are issued per chunk on the GpSimd SWDGE ring (and the last
    chunk's halves on the SP ring tail) so they overlap the input stream
    instead of queueing behind it.
  - Phase-1 runs weight-major (all W-top matmuls of a chunk, then all
    W-bottom) to amortize LDWEIGHTS.
  - PSUM->SBUF traffic is spread over ACT (C rows), DVE (D rows) and
    ACT/DVE/Pool round-robin for the output casts.
Inputs/weights are fp16 on device; accumulation is fp32 in PSUM; output is
staged fp16 and upcast to fp32 on host (rel err ~7e-4 end to end).
Sharding: batch b -> core b (8 cores, no communication).
"""

import os
import sys
import types

import numpy as np

import concourse.bass as bass
import concourse.bacc as bacc
import concourse.tile as tile
from concourse import mybir
from concourse.bass_utils import run_bass_kernel_spmd

B, M, N, I = 8, 256, 256, 32
KMAX = 16           # modes kept: 0..15
R = 2 * KMAX - 1    # 31 real basis vectors
FREE = I * N        # 8192
CCOLS = 1024        # max chunk width (4 channels)
F32 = mybir.dt.float32
F16 = mybir.dt.float16
F8 = mybir.dt.float8e4
NPDT = np.float16

WIDTHS = [512] + [1024] * 7 + [512]

LAST_RESULTS = None  # BassKernelResults of the most recent run (for test.py)


def _ensure_ntff_hook():
    """Provide antenv.axon_hooks if the image lacks it, so trace=True works."""
    try:
        from antenv.axon_hooks import get_axon_ntff_profile_hook  # noqa: F401
        return
    except ImportError:
        pass
    try:
        from trn_agent_boot.trn_boot import _ntff_profile_via_ctypes
        hook = _ntff_profile_via_ctypes("/opt/axon/libaxon_pjrt.so")
    except Exception:
        hook = None
    mod = types.ModuleType("antenv.axon_hooks")
    _state = {"hook": hook}
    mod.get_axon_ntff_profile_hook = lambda: _state["hook"]
    mod.set_axon_ntff_profile_hook = lambda h: _state.update(hook=h)
    sys.modules["antenv.axon_hooks"] = mod
    try:
        import antenv
        antenv.axon_hooks = mod
    except ImportError:
        pass


def _basis():
    t = np.arange(N)
    cols = [np.ones(N) / np.sqrt(N)]
    for k in range(1, KMAX):
        cols.append(np.sqrt(2.0 / N) * np.cos(2 * np.pi * k * t / N))
        cols.append(-np.sqrt(2.0 / N) * np.sin(2 * np.pi * k * t / N))
    return np.stack(cols, axis=1).astype(np.float32)  # [256, 31]


def _build_nc():
    nc = bacc.Bacc("TRN2", target_bir_lowering=False, debug=False,
                   enable_asserts=False, num_devices=8)

    # xft stacks the channel-major layout (rows 0:256) and the host-side
    # transposed layout (rows 256:512) in ONE dram tensor so each chunk's
    # x and t arrive with a single DMA instruction (descriptor-gen on the
    # issuing sequencer is ~0.7us per instruction -- it paces the early
    # pipeline, so fewer+fatter instructions win)
    xft = nc.declare_dram_parameter("xft", [2 * M, FREE], F16, isOutput=False)
    w2 = nc.declare_dram_parameter("w2", [128, 2 * R], F16, isOutput=False)
    wz = nc.declare_dram_parameter("wz", [R + 1, CCOLS], F16, isOutput=False)
    zw = nc.declare_dram_parameter("zw", [R + 1, CCOLS], F16, isOutput=False)
    out = nc.declare_dram_parameter("out", [M, FREE], F16, isOutput=True)

    starts = [0]
    for w_ in WIDTHS[:-1]:
        starts.append(starts[-1] + w_)

    with tile.TileContext(nc) as tc:
        with (
            tc.tile_pool(name="const", bufs=1) as constp,
            tc.tile_pool(name="xin", bufs=len(WIDTHS)) as xin,
            tc.tile_pool(name="oput", bufs=len(WIDTHS)) as outp,
            tc.tile_pool(name="pcd", bufs=3, space=bass.MemorySpace.PSUM) as pcdp,
            tc.tile_pool(name="p2", bufs=5, space=bass.MemorySpace.PSUM) as p2p,
        ):
            w2sb = constp.tile([128, 2 * R], F16)
            nc.sync.dma_start(out=w2sb[:], in_=w2[:])

            # Each L/R buffer has a low half (partitions 0:63) and a high
            # half (64:127): consecutive f-tiles alternate halves so their
            # phase-2 K=63 matmuls co-issue on the two PE row halves (the
            # row tile position must equal the stationary's start partition).
            NLR = 2
            Lgs = [constp.tile([128, CCOLS], F16, name=f"Lg{k}")
                   for k in range(NLR)]
            Rgs = [constp.tile([128, CCOLS], F16, name=f"Rg{k}")
                   for k in range(NLR)]

            # all input DMAs are issued up front so the SP/ACT sequencers
            # never block input descriptor-gen behind an output that is
            # waiting on casts. Chunk 0 (x and t) rides SP first so the PE
            # starts ASAP; outputs go on the SP ring tail, FIFO behind all
            # inputs = strict input priority on HBM bandwidth.
            # one input tile per chunk: [:, 0:2, :] = x halves, [:, 2:4, :]
            # = t halves
            xtgs = [xin.tile([128, 4, w], F16, tag="x", name=f"xtg{g}")
                    for g, w in enumerate(WIDTHS)]
            # NOTE: the GpSimd SWDGE queue is fed by software descriptor-gen
            # and, while active, drags the whole 16-engine DMA pool down to
            # ~250GB/s (measured) -- never put anything on it. Everything
            # rides ONE HWDGE queue (SP) in hand-crafted FIFO order: the
            # FIFO then delivers each chunk's x+t just-in-time, consts early,
            # and drains each chunk's output right behind the input of the
            # chunk two ahead -- strict pipeline pacing with zero cross-queue
            # competition for the 16 DMA engines.
            def load_in(g):
                c0, w = starts[g], WIDTHS[g]
                nc.sync.dma_start(
                    out=xtgs[g][:],
                    in_=xft[:, c0:c0 + w].rearrange("(q p) c -> p q c", q=4))

            IN_LEAD = 4  # chunks of input issued ahead of the pipeline
            load_in(0)
            load_in(1)
            # consts interleave between early input chunks, ordered by
            # first use (f-tile s uses buffer chunk(s)%2, half s%2)
            const_seq = [(0, 0), (1, 64), (1, 0), (0, 64)]
            for g in range(2, IN_LEAD):
                for k, h in const_seq[(g - 2) * 2:(g - 1) * 2]:
                    nc.sync.dma_start(out=Lgs[k][h:h + 32, :], in_=wz[:])
                    nc.sync.dma_start(out=Rgs[k][h + 31:h + 63, :], in_=zw[:])
                load_in(g)

            ogs = [outp.tile([128, 2, w], F16, tag="o", name=f"og{g}")
                   for g, w in enumerate(WIDTHS)]

            # global f-tile pipeline: each f-tile is 512 cols = one channel
            # pair; phase 2 of pair s runs two f-tiles behind phase 1 of
            # pair s. tile_set_cur_wait pins this order in the scheduler so
            # the in-order PE never queues input-blocked phase-1 work ahead
            # of ready phase-2 work.
            fl = []
            for g, (c0, w) in enumerate(zip(starts, WIDTHS)):
                for f in range(w // 512):
                    fl.append((g, f))

            def do_p1_pair(ftiles):
                # two f-tiles share one [128, 512] PSUM bank: C1@0, D1@32,
                # C2@64, D2@96 -- explicit tile_position unlocks the 96-base
                # quadrant so 4 PE column groups stream concurrently.
                pc = pcdp.tile([128, 512], F32, tag="pcd", name="pc")
                units = []
                for u, (g, f) in enumerate(ftiles):
                    fsl = slice(f * 512, (f + 1) * 512)
                    units.append((64 * u, g, fsl))
                for half, W_cols in ((0, slice(0, R)), (1, slice(R, 2 * R))):
                    st = half == 0
                    sp = half == 1
                    for r0, g, fsl in units:
                        nc.tensor.matmul(pc[r0:r0 + R, :], w2sb[:, W_cols],
                                         xtgs[g][:, half, fsl],
                                         start=st, stop=sp,
                                         tile_position=(0, r0))
                        nc.tensor.matmul(pc[r0 + 32:r0 + 63, :],
                                         w2sb[:, W_cols],
                                         xtgs[g][:, 2 + half, fsl],
                                         start=st, stop=sp,
                                         tile_position=(0, r0 + 32))
                for u, (r0, g, fsl) in enumerate(units):
                    Lg, Rg = Lgs[g % NLR], Rgs[g % NLR]
                    hb = 64 * u  # L/R half for this f-tile (pair parity)
                    # C rows on ACT, D rows on DVE
                    nc.scalar.copy(Rg[hb:hb + R, fsl], pc[r0:r0 + R, :])
                    nc.vector.tensor_copy(Lg[hb + 32:hb + 63, fsl],
                                          pc[r0 + 32:r0 + 63, :])

            cast_engines = [lambda o, i: nc.vector.tensor_copy(o, i),
                            lambda o, i: nc.scalar.copy(o, i)]

            def ship(g):
                # whole chunk cast: ship it, then queue the input of the
                # chunk IN_LEAD ahead right behind it in the FIFO
                c0, w = starts[g], WIDTHS[g]
                dst = out[:, c0:c0 + w].rearrange("(h p) c -> p h c", h=2)
                nc.sync.dma_start(out=dst, in_=ogs[g][:])
                if g + IN_LEAD < len(WIDTHS):
                    load_in(g + IN_LEAD)

            def do_p2_pair(ftiles):
                # the two f-tiles' K=63 matmuls co-issue on PE row halves
                # (tile_position row base 0 / 64)
                for j in range(2):
                    p2s = []
                    for u, (g, f) in enumerate(ftiles):
                        p2 = p2p.tile([128, 2 * N], F32, tag="p2", name="p2")
                        p2s.append(p2)
                    for s in range(2):
                        for u, (g, f) in enumerate(ftiles):
                            Lg, Rg = Lgs[g % NLR], Rgs[g % NLR]
                            hb = 64 * u
                            il = 2 * f + s
                            csl = slice(il * N, (il + 1) * N)
                            jsl = slice(il * N + j * 128,
                                        il * N + (j + 1) * 128)
                            nc.tensor.matmul(p2s[u][:, s * N:(s + 1) * N],
                                             Lg[hb:hb + 63, jsl],
                                             Rg[hb:hb + 63, csl],
                                             start=True, stop=True)
                    for u, (g, f) in enumerate(ftiles):
                        cast_engines[(j + u) % 2](
                            ogs[g][:, j, 2 * f * N:(2 * f + 2) * N], p2s[u][:])
                for g, f in ftiles:
                    if f == WIDTHS[g] // 512 - 1:
                        ship(g)

            # phase-1 advances in f-tile pairs; phase-2 lags one pair
            groups = [fl[i:i + 2] for i in range(0, len(fl), 2)]
            for q in range(len(groups) + 1):
                tc.tile_set_cur_wait(0.01 * (q + 1))
                if q < len(groups):
                    do_p1_pair(groups[q])
                if q >= 1:
                    tc.tile_set_cur_wait(0.01 * (q + 1) + 0.005)
                    do_p2_pair(groups[q - 1])

    nc.finalize()
    return nc


_NC = None


def kernel(x: np.ndarray) -> np.ndarray:
    global _NC, LAST_RESULTS
    x = np.asarray(x)
    assert x.shape == (B, M, N, I), x.shape

    W = _basis().astype(NPDT)          # [256, 31]
    Wt = W.T.copy()                    # [31, 256]
    w2_np = np.concatenate([W[0:128, :], W[128:256, :]], axis=1)  # [128, 62]
    wtile = np.tile(Wt, (1, CCOLS // N))                          # [31, CCOLS]
    wz_np = np.concatenate([wtile, np.zeros((1, CCOLS), NPDT)], axis=0)
    zw_np = np.concatenate([np.zeros((1, CCOLS), NPDT), wtile], axis=0)

    if _NC is None:
        _NC = _build_nc()

    xq = np.asarray(x, dtype=NPDT)
    in_maps = []
    for b in range(B):
        xcm = np.ascontiguousarray(xq[b].transpose(0, 2, 1)).reshape(M, FREE)
        xtm = np.ascontiguousarray(xq[b].transpose(1, 2, 0)).reshape(N, I * M)
        in_maps.append({
            "xft": np.concatenate([xcm, xtm], axis=0),
            "w2": w2_np, "wz": wz_np, "zw": zw_np,
        })

    trace = bool(int(os.environ.get("KERNEL_TRACE", "0")))
    if trace:
        _ensure_ntff_hook()
    last_err = None
    for attempt in range(3):
        try:
            LAST_RESULTS = run_bass_kernel_spmd(_NC, in_maps, list(range(B)),
                                                trace=trace and attempt == 0)
            break
        except Exception as e:  # rare transient NRT_EXEC_UNIT_UNRECOVERABLE
            last_err = e
            import time as _time
            _time.sleep(2.0)
            try:
                import jax
                jax.clear_caches()
                jax.extend.backend.clear_backends()
            except Exception:
                pass
    else:
        raise last_err

    out = np.empty((B, M, N, I), np.float32)
    for b in range(B):
        dev = LAST_RESULTS.results[b]["out"].astype(np.float32).reshape(M, I, N)
        out[b] = dev.transpose(0, 2, 1)
    return out
